# revision 6
# baseline (speedup 1.0000x reference)
"""Self-contained Trainium2 (Bass/Tile) kernel for nn_CQAttention.

kernel(**inputs) takes FULL inputs (B=64) and returns the FULL output
[64, 2048, 512] (= concat[C, A, C*A, C*Bm]). Internally shards batch across
8 NeuronCores (data parallel, 8 batches/core) and runs a Bass/Tile program
via concourse.bass_utils.run_bass_kernel_spmd.

Math (per batch; bias is a constant shift so it cancels in both softmaxes).
Factor E = exp(S) = exp(s2) * e^{s0[c]} * e^{s1[q]} so the similarity
matmuls need no rank-2 updates and the exponentials are pure exp(s2):
  F    = exp(s2)         (both layouts: ec=[c,q] tiles, eq=[q,c] tiles)
  f    = e^{s0} [c],  g = e^{s1} [q]
  M_aug= F^T [f*C | f] = [M | s]          (s = colsum(f*F) = S2 normalizer)
  A_aug= F [g*Q | g | (g/s)*M] = [A_raw | r | Bm_raw]   (r = rowsum(F*g))
  A = A_raw / r;  Bm = Bm_raw / r
  out = [C, A, C*A, C*Bm]

All PSUM lives in one [128,2,512] x4 ring (8 banks): transpose staging,
pair-exps (2 S-tiles per activation), the M accumulator, and the 257-wide
A|r|Bm output pairs. PSUM matmul accumulation chains are kept contiguous
on the PE queue (interleaving them corrupts results on HW).
"""
import sys
import numpy as np

for _p in ("/opt/trn_rl_repo",):
    if _p not in sys.path:
        sys.path.insert(0, _p)

import concourse.bass as bass
import concourse.mybir as mybir
import concourse.tile as tile
from concourse import bacc
from concourse.masks import make_identity
from concourse.bass_utils import run_bass_kernel_spmd
from contextlib import ExitStack

F32 = mybir.dt.float32
F32R = mybir.dt.float32r
BF16 = mybir.dt.bfloat16
AF = mybir.ActivationFunctionType
AX = mybir.AxisListType

N_CORES = 8
B, CL, QL, D = 64, 2048, 512, 128
NB = B // N_CORES  # batches per core


def _build_body(nc, tc, ctx, nb, cl, ql, d, C_d, Q_d, w4c_d, w4q_d, w4m_d, OUT_d):
    NT = cl // 128   # 16 c-tiles
    NQ = ql // 128   # 4 q-tiles

    consts = ctx.enter_context(tc.tile_pool(name="consts", bufs=1))
    ident = consts.tile([128, 128], F32)
    make_identity(nc, ident)
    w4c_sb = consts.tile([d, 1], F32)
    nc.sync.dma_start(w4c_sb, w4c_d)
    w4q_sb = consts.tile([d, 1], F32)
    nc.sync.dma_start(w4q_sb, w4q_d)
    w4m_sb = consts.tile([d, 1], F32)
    nc.sync.dma_start(w4m_sb, w4m_d)
    # fp32r matmul operands must be produced as float32r (rounded).
    # Width 2 (duplicated column): fp32r matmuls need even moving/dst sizes.
    w4c_r = consts.tile([d, 2], F32R)
    nc.vector.tensor_copy(w4c_r, w4c_sb.broadcast_to((d, 2)))
    w4q_r = consts.tile([d, 2], F32R)
    nc.vector.tensor_copy(w4q_r, w4q_sb.broadcast_to((d, 2)))
    ones_bf = consts.tile([128, NT], BF16)
    ones_f = consts.tile([128, NT], F32)
    nc.gpsimd.memset(ones_f, 1.0)
    nc.vector.tensor_copy(ones_bf, ones_f)

    ld = ctx.enter_context(tc.tile_pool(name="ld", bufs=2))
    ctp = ctx.enter_context(tc.tile_pool(name="ctp", bufs=2))
    small = ctx.enter_context(tc.tile_pool(name="small", bufs=2))
    epool = ctx.enter_context(tc.tile_pool(name="epool", bufs=2))
    outp = ctx.enter_context(tc.tile_pool(name="outp", bufs=3))
    csp = ctx.enter_context(tc.tile_pool(name="csp", bufs=3))

    u_ps = ctx.enter_context(tc.tile_pool(name="u_ps", bufs=4, space="PSUM"))

    def uslot(name):
        return u_ps.tile([128, 2, 512], F32, tag="u", name=name)

    def bcast(ap2d, n):
        # [128, k] -> [128, k, n] with a step-0 inner dim
        return ap2d.unsqueeze(2).broadcast_to((128, ap2d.shape[1], n))

    for b in range(nb):
        # ---- loads ----
        c_nat = ld.tile([128, NT, d], F32, tag="c_nat")
        nc.sync.dma_start(c_nat, C_d[b].rearrange("(t p) d -> p t d", p=128))
        q_nat = ld.tile([128, NQ, d], F32, tag="q_nat")
        nc.sync.dma_start(q_nat, Q_d[b].rearrange("(t p) d -> p t d", p=128))

        # ---- PE transposes, 8 per psum slot ----
        ct_t = ctp.tile([128, NT, d], F32R, tag="ct")    # [d, t, c]
        for g in range(2):
            tp = uslot("tp").rearrange("p a (b c) -> p (a b) c", c=128)
            for i in range(8):
                nc.tensor.transpose(tp[:, i, :], c_nat[:, g * 8 + i, :], ident)
            nc.vector.tensor_copy(ct_t[:, g * 8:(g + 1) * 8, :], tp)
        # q transposes + s0/s1 matvecs share one slot
        tq = uslot("tq")
        tq_flat = tq.rearrange("p a b -> p (a b)")
        tq4 = tq_flat[:, 0:512].rearrange("p (a b) -> p a b", b=128)
        for i in range(NQ):
            nc.tensor.transpose(tq4[:, i, :], q_nat[:, i, :], ident)
        qt_t = small.tile([128, NQ, d], F32R, tag="qt")  # [d, qt, q]
        nc.vector.tensor_copy(qt_t, tq4)

        ct_flat = ct_t.rearrange("p a b -> p (a b)")     # [d, cl]
        qt_flat = qt_t.rearrange("p a b -> p (a b)")     # [d, ql]

        qwt = small.tile([128, ql], F32R, tag="qwt")     # QT * w4mul
        nc.vector.tensor_scalar_mul(qwt, qt_flat, w4m_sb)

        # s0 = C@w4c in [c-part, NT], s1 = Q@w4q in [q-part, NQ]
        # (pair-width fp32r matvecs into cols 512.. of the tq slot)
        sv2 = tq_flat[:, 512:512 + 2 * (NT + NQ)].rearrange(
            "p (a c) -> p a c", c=2)
        for t in range(NT):
            nc.tensor.matmul(sv2[:, t, :], ct_t[:, t, :], w4c_r)
        for qi in range(NQ):
            nc.tensor.matmul(sv2[:, NT + qi, :], qt_t[:, qi, :], w4q_r)
        f_sb = small.tile([128, NT], F32, tag="f")       # f = e^{s0}
        nc.scalar.activation(f_sb, sv2[:, 0:NT, 0], AF.Exp)
        g_sb = small.tile([128, NQ], F32, tag="g")       # g = e^{s1}
        nc.scalar.activation(g_sb, sv2[:, NT:NT + NQ, 0], AF.Exp)

        # ---- Cf_aug = [f*C | f] bf16 (mul on Pool engine) ----
        cf_aug = small.tile([128, NT, d + 1], BF16, tag="cf_aug")
        nc.gpsimd.tensor_mul(cf_aug[:, :, 0:d], c_nat, bcast(f_sb, d))
        nc.vector.tensor_copy(cf_aug[:, :, d:d + 1], f_sb.unsqueeze(2))

        # ---- rhs_cat = [g*Q | g | (g/s)*M]  [q-part, NQ, 257] bf16 ----
        rhs_cat = small.tile([128, NQ, 2 * d + 1], BF16, tag="rhs_cat")
        nc.vector.tensor_mul(rhs_cat[:, :, 0:d], q_nat, bcast(g_sb, d))
        nc.vector.tensor_copy(rhs_cat[:, :, d:d + 1], g_sb.unsqueeze(2))

        # ---- F pass in [c,q] layout: 2 S-tiles per slot, one exp per pair
        ec = epool.tile([128, NT, ql], BF16, tag="ec")
        for t in range(0, NT, 2):
            ep = uslot("ep")
            nc.tensor.matmul(ep[:, 0, :], ct_t[:, t, :], qwt)
            nc.tensor.matmul(ep[:, 1, :], ct_t[:, t + 1, :], qwt)
            nc.scalar.activation(ec[:, t:t + 2, :], ep, AF.Exp)

        # ---- M_aug = F^T [f*C | f] -> [M | s].  Accumulation chains stay
        # contiguous on the PE queue (interleaving corrupts them on HW).
        mm = uslot("mm").rearrange("p a (b c) -> p (a b) c", c=256)
        for qi in range(NQ):
            for t in range(NT):
                nc.tensor.matmul(mm[:, qi, 0:d + 1],
                                 ec[:, t, qi * 128:(qi + 1) * 128],
                                 cf_aug[:, t, :],
                                 start=(t == 0), stop=(t == NT - 1))
        sinv = small.tile([128, NQ], F32, tag="sinv")
        nc.vector.reciprocal(sinv, mm[:, :, d])
        gs = small.tile([128, NQ], F32, tag="gs")
        nc.vector.tensor_mul(gs, g_sb, sinv)
        nc.vector.tensor_mul(rhs_cat[:, :, d + 1:2 * d + 1], mm[:, :, 0:d],
                             bcast(gs, d))

        # ---- F^T pass in [q,c] layout ----
        eq = epool.tile([128, NQ, cl], BF16, tag="eq")
        for t in range(0, NT, 2):
            qi, j = t // 4, t % 4
            ep2 = uslot("ep2")
            nc.tensor.matmul(ep2[:, 0, :], qwt[:, qi * 128:(qi + 1) * 128],
                             ct_flat[:, j * 512:(j + 1) * 512])
            nc.tensor.matmul(ep2[:, 1, :], qwt[:, qi * 128:(qi + 1) * 128],
                             ct_flat[:, (j + 1) * 512:(j + 2) * 512])
            nc.scalar.activation(eq[:, qi, j * 512:(j + 2) * 512],
                                 ep2.rearrange("p a b -> p (a b)"), AF.Exp)

        # ---- A_aug pairs: [A_raw | r | Bm_raw] per c-tile ----
        out_r = OUT_d[b].rearrange("(t p) n -> p t n", p=128)
        for pr in range(NT // 2):
            ab = uslot("ab")
            for i in range(2):
                t = pr * 2 + i
                for qi in range(NQ):
                    nc.tensor.matmul(ab[:, i, 0:2 * d + 1],
                                     eq[:, qi, t * 128:(t + 1) * 128],
                                     rhs_cat[:, qi, :],
                                     start=(qi == 0), stop=(qi == NQ - 1))
            rr = small.tile([128, 2], F32, tag="rr")
            nc.vector.reciprocal(rr, ab[:, :, d])
            cs2 = csp.tile([128, 2, 128], F32, tag="cs")
            nc.vector.tensor_mul(cs2, c_nat[:, pr * 2:pr * 2 + 2, :],
                                 bcast(rr, d))
            ob = outp.tile([128, 2, 384], F32, tag="ob")
            nc.vector.tensor_mul(ob[:, :, 0:128], ab[:, :, 0:d], bcast(rr, d))
            nc.vector.tensor_mul(ob[:, :, 128:256], ab[:, :, 0:d], cs2)
            nc.vector.tensor_mul(ob[:, :, 256:384], ab[:, :, d + 1:2 * d + 1],
                                 cs2)
            nc.gpsimd.dma_start(out_r[:, pr * 2:pr * 2 + 2, 128:512], ob)
            if pr % 2 == 0:
                nc.sync.dma_start(out_r[:, pr * 2:pr * 2 + 4, 0:128],
                                  c_nat[:, pr * 2:pr * 2 + 4, :])


def build_program(nb=NB):
    nc = bacc.Bacc("TRN2", target_bir_lowering=False, debug=False,
                   num_devices=N_CORES)
    C_d = nc.dram_tensor("C", [nb, CL, D], F32, kind="ExternalInput").ap()
    Q_d = nc.dram_tensor("Q", [nb, QL, D], F32, kind="ExternalInput").ap()
    w4c_d = nc.dram_tensor("w4c", [D, 1], F32, kind="ExternalInput").ap()
    w4q_d = nc.dram_tensor("w4q", [D, 1], F32, kind="ExternalInput").ap()
    w4m_d = nc.dram_tensor("w4mul", [D, 1], F32, kind="ExternalInput").ap()
    OUT_d = nc.dram_tensor("OUT", [nb, CL, 4 * D], F32, kind="ExternalOutput").ap()
    with ExitStack() as ctx:
        tc = ctx.enter_context(tile.TileContext(nc))
        _build_body(nc, tc, ctx, nb, CL, QL, D,
                    C_d, Q_d, w4c_d, w4q_d, w4m_d, OUT_d)
    nc.compile()
    return nc


_PROGRAM_CACHE = {}


def _get_program(nb=NB):
    if nb not in _PROGRAM_CACHE:
        _PROGRAM_CACHE[nb] = build_program(nb)
    return _PROGRAM_CACHE[nb]


def _numpy_fallback(C, Q, c_mask, q_mask, w4c, w4q, w4mul, bias):
    """Exact reference math in numpy (used only if masks are not all-ones)."""
    NEG_INF = -1e30
    out = np.empty((C.shape[0], C.shape[1], 4 * C.shape[2]), np.float32)
    for b in range(C.shape[0]):
        Cb = C[b].astype(np.float64)
        Qb = Q[b].astype(np.float64)
        S = (Cb @ w4c.reshape(-1, 1) + (Qb @ w4q.reshape(-1, 1)).T
             + (Cb * w4mul.reshape(1, -1)) @ Qb.T + float(np.asarray(bias).reshape(-1)[0]))
        qm = q_mask[b].reshape(1, -1)
        cm = c_mask[b].reshape(-1, 1)
        S1l = S * qm + NEG_INF * (1.0 - qm)
        S2l = S * cm + NEG_INF * (1.0 - cm)
        S1 = np.exp(S1l - S1l.max(1, keepdims=True))
        S1 /= S1.sum(1, keepdims=True)
        S2 = np.exp(S2l - S2l.max(0, keepdims=True))
        S2 /= S2.sum(0, keepdims=True)
        A = S1 @ Qb
        Bm = S1 @ (S2.T @ Cb)
        out[b] = np.concatenate([Cb, A, Cb * A, Cb * Bm], axis=1).astype(np.float32)
    return out


def kernel(C, Q, c_mask, q_mask, w4c, w4q, w4mul, bias):
    C = np.ascontiguousarray(np.asarray(C), dtype=np.float32)
    Q = np.ascontiguousarray(np.asarray(Q), dtype=np.float32)
    c_mask = np.asarray(c_mask)
    q_mask = np.asarray(q_mask)
    w4c = np.asarray(w4c, dtype=np.float32)
    w4q = np.asarray(w4q, dtype=np.float32)
    w4mul = np.asarray(w4mul, dtype=np.float32)

    if not (np.all(c_mask == 1.0) and np.all(q_mask == 1.0)):
        return _numpy_fallback(C, Q, c_mask, q_mask, w4c, w4q, w4mul, bias)

    nc = _get_program(NB)
    w4c_r = np.ascontiguousarray(w4c.reshape(D, 1))
    w4q_r = np.ascontiguousarray(w4q.reshape(D, 1))
    w4m_r = np.ascontiguousarray(w4mul.reshape(D, 1))
    in_maps = []
    for c in range(N_CORES):
        sl = slice(c * NB, (c + 1) * NB)
        in_maps.append({
            "C": np.ascontiguousarray(C[sl]),
            "Q": np.ascontiguousarray(Q[sl]),
            "w4c": w4c_r,
            "w4q": w4q_r,
            "w4mul": w4m_r,
        })
    res = run_bass_kernel_spmd(nc, in_maps, core_ids=list(range(N_CORES)))
    out = np.concatenate([res.results[c]["OUT"] for c in range(N_CORES)], axis=0)
    return out
